# revision 38
# baseline (speedup 1.0000x reference)
"""Trainium2 Bass kernel for equivariant multihead attention.

Math (per batch b, query point i, coset s1, channel c):
    logit[j,s2] = sum_g pairwise_g[b,i,j,s1,s2,g]*w_g[c,g]
                  + w_y[c,0]*y[b,j,s2,c] + w_y[c,1]*y[b,i,s1,c] + b_g[c] + b_y[c]
    att = exp(logit)*mask[b,j,s2];  att /= sum_{j,s2} att
    out = (y[b,i,s1,c] + sum_{j,s2} att*y[b,j,s2,c]) * mask[b,i,s1]  @ w_lin.T

The query-side term and the biases are constant over the key dims (j,s2), so
they cancel in the normalization and are dropped.  The key-side factor
exp(w_y[c,0]*y[b,j,s2,c])*mask[b,j,s2] is a tiny per-batch table KD (and
KD*y = KN), precomputed on host.  Per (b,i) block the device computes
    E[(s1,s2,c), j] = exp(sum_g G_T[(s1,s2,g), j] * w_g[c,g])
    den_part[(s1,s2,c)] = sum_j E * KD_T     (fused multiply-reduce)
    num_part[(s1,s2,c)] = sum_j E * KN_T
and one final PE matmul sums the partials over s2.  Host finishes with the
residual add, query mask, and the c_in->c_out linear (all tiny).

Sharding: query dim i is split 8 ways (16 i x 4 b = 64 blocks per core).

Implementation notes (walrus on this stack allows only ONE sync wait per
Matmult / DMA / STT instruction, and ~12 on the final drain):
  * ALL inputs ship as ONE dram "blob" per core, loaded by 7 big
    column-range DMAs into a single SBUF plane -> every DMA is the first
    on its HW-DGE queue (no proc-predecessor wait) and descriptor runs are
    ~17KB contiguous (max DMA efficiency).  The final store is the 8th DMA
    (queue 7, also virgin).
  * tiny "spacer" ops make each engine observe cross-engine ticks ahead of
    the real instructions, so those carry at most one wait each.
"""

import numpy as np

import concourse.bacc as bacc
import concourse.tile as tile
from concourse import mybir
from concourse.bass_utils import run_bass_kernel_spmd

B, N, S, CIN, COUT, GDIM = 4, 128, 8, 8, 8, 7
NCORES = 8
ISHARD = N // NCORES          # 16 query points per core
NBLK = B * ISHARD             # 64 (b,i) blocks per core
PW = S * GDIM                 # 56: free width of one s1 slice
QW = 2 * PW                   # 112: free width of one transpose quarter
NQ = 4                        # quarters per block
BW = NQ * QW                  # 448 floats per (j, block)
NCOL = NBLK * NQ              # 256 partial columns per half

# blob column layout: [ident | kd | kn | bd | sind | pg blocks]
IDENT0 = 0
KD0 = 128
KN0 = KD0 + B * N             # 640
BD0 = KN0 + B * N             # 1152
SIND0 = BD0 + 128             # 1280
CONSTW = SIND0 + 16           # 1296
TOTW = CONSTW + NBLK * BW     # 29968

# blocks covered by each of the 7 input DMAs (first also carries consts;
# earlier ones smaller for a faster pipeline ramp)
SUPER_BLOCKS = (4, 6, 8, 10, 11, 12, 13)

F32 = mybir.dt.float32

_PROGRAM_CACHE = {}


def _build_program(nblk=NBLK):
    nc = bacc.Bacc("TRN2", target_bir_lowering=False, debug=False,
                   num_devices=NCORES)

    blob_d = nc.dram_tensor("blob", (N, TOTW), F32, kind="ExternalInput").ap()
    out_s = nc.dram_tensor("out_s", (16, 2 * NCOL), F32,
                           kind="ExternalOutput").ap()

    # per-super [start_block, end_block) and column ranges
    supers = []
    blk0 = 0
    for nb in SUPER_BLOCKS:
        if blk0 >= nblk:
            break
        nb = min(nb, nblk - blk0)
        c0 = 0 if blk0 == 0 else CONSTW + blk0 * BW
        c1 = CONSTW + (blk0 + nb) * BW
        supers.append((blk0, blk0 + nb, c0, c1))
        blk0 += nb

    with tile.TileContext(nc) as tc:
        with (
            tc.tile_pool(name="consts", bufs=1) as consts,
            tc.tile_pool(name="gtpool", bufs=3) as gtpool,
            tc.tile_pool(name="epool", bufs=3) as epool,
            tc.tile_pool(name="psA", bufs=3, space="PSUM") as psA,
            tc.tile_pool(name="psB", bufs=2, space="PSUM") as psB,
            tc.tile_pool(name="psC", bufs=1, space="PSUM") as psC,
            tc.tile_pool(name="psJ", bufs=1, space="PSUM") as psJ,
        ):
            g_all = consts.tile([N, TOTW], F32)
            ident = g_all[:, IDENT0:IDENT0 + 128]
            bd = g_all[0:QW, BD0:BD0 + 128]
            sind = g_all[:, SIND0:SIND0 + 16]

            for (b0, b1, c0, c1) in supers:
                nc.sync.dma_start(g_all[:, c0:c1], blob_d[:, c0:c1])

            buf_dve = consts.tile([128, 2 * NCOL], F32)
            nc.vector.memset(buf_dve, 0.0)
            # DVE observes the super-0 queue (kd/kn/consts) once
            junk_d0 = consts.tile([128, 1], F32)
            nc.vector.tensor_copy(junk_d0, g_all[:, KD0:KD0 + 1])

            NDUM = 8
            dummies = [consts.tile([128, 1], F32, name=f"dum{i}")
                       for i in range(NDUM)]
            dum_idx = [0]
            junk_sp = consts.tile([128, NBLK], F32)
            fence_buf = consts.tile([128, 2, NQ * NBLK], F32)
            junk_act = consts.tile([128, NBLK], F32)
            junk_act2 = consts.tile([128, NBLK], F32)
            s_sb = consts.tile([16, 2 * NCOL], F32)

            junk_ps = psJ.tile([1, 128], F32)

            def stt_reduce(e_q, table, col_ap):
                dum = dummies[dum_idx[0] % NDUM]
                dum_idx[0] += 1
                nc.vector.scalar_tensor_tensor(
                    dum.broadcast_to(e_q.shape), e_q, 0.0, table,
                    op0=mybir.AluOpType.bypass, op1=mybir.AluOpType.mult,
                    accum_out=col_ap)

            for (b0, b1, c0, c1) in supers:
                # PE observes this super's DMA queue once; real transposes
                # then carry only their PSUM-slot WAR wait.
                nc.tensor.transpose(junk_ps, g_all[:, c0:c0 + 1], ident)
                for blk in range(b0, b1):
                    b = blk // ISHARD
                    gcol = CONSTW + blk * BW
                    kd_b = g_all[:, KD0 + b * N:KD0 + (b + 1) * N]
                    kn_b = g_all[:, KN0 + b * N:KN0 + (b + 1) * N]

                    gt_cat = gtpool.tile([QW, NQ, 128], F32, tag="gt")
                    for q in range(NQ):
                        gt_ps = psA.tile([QW, 128], F32, tag="gtps")
                        nc.tensor.transpose(
                            gt_ps,
                            g_all[:, gcol + QW * q:gcol + QW * (q + 1)],
                            ident)
                        nc.scalar.copy(gt_cat[:, q, :], gt_ps)

                    l_ps = psB.tile([128, NQ, 128], F32, tag="lps")
                    nc.tensor.matmul(l_ps, lhsT=bd, rhs=gt_cat,
                                     start=True, stop=True)
                    # ACT self-spacer: emits the {ACT >= copies(blk)} tick so
                    # the block reusing this gt_cat slot later carries only
                    # its {PE} wait.
                    nc.scalar.copy(junk_act2[:QW, blk:blk + 1],
                                   gt_cat[:, NQ - 1, 127:128])

                    e_t = epool.tile([128, NQ, 128], F32, tag="e")
                    nc.scalar.activation(e_t, l_ps,
                                         mybir.ActivationFunctionType.Exp)
                    # DVE spacer: absorbs the {ACT exp} wait for the stts
                    nc.vector.tensor_copy(junk_sp[:, blk:blk + 1],
                                          e_t[:, 0, 0:1])

                    for q in range(NQ):
                        col = blk * NQ + q
                        e_q = e_t[:, q, :]
                        stt_reduce(e_q, kd_b, buf_dve[:, col:col + 1])
                        stt_reduce(e_q, kn_b,
                                   buf_dve[:, NCOL + col:NCOL + col + 1])

                    # fence: data-ordered after all 8 accum writes; the ACT
                    # read of it makes ACT observe the DVE tick, so the next
                    # exp reusing this e-slot has no DVE WAR wait.
                    bcols = buf_dve.rearrange("p (h c) -> p h c", h=2)
                    nc.vector.tensor_copy(
                        fence_buf[:, :, NQ * blk:NQ * (blk + 1)],
                        bcols[:, :, NQ * blk:NQ * (blk + 1)])
                    nc.scalar.copy(junk_act[:, blk:blk + 1],
                                   fence_buf[:, 0, NQ * blk:NQ * blk + 1])

            # sum the (h,s2,c) j-partials over s2 -> (h,c)
            s_ps = psC.tile([16, 2 * NCOL], F32)
            nc.tensor.matmul(s_ps, lhsT=sind, rhs=buf_dve,
                             start=True, stop=True)
            nc.scalar.copy(s_sb, s_ps)
            nc.sync.dma_start(out_s, s_sb)   # 8th DMA -> virgin queue 7

    nc.compile()   # bacc: register alloc + split_sync_waits (1-wait limit)
    return nc


def _get_program(nblk=NBLK):
    key = ("nc", nblk)
    if key not in _PROGRAM_CACHE:
        _PROGRAM_CACHE[key] = _build_program(nblk)
    return _PROGRAM_CACHE[key]


def _host_prep(pairwise_g, coset_functions, mask, w_y, w_g):
    """Build the per-core input blobs."""
    y = coset_functions.astype(np.float32)          # (B, N, S, C) keys
    maskf = mask.astype(np.float32)
    ey = np.exp(y * w_y[:, 0]) * maskf[..., None]   # (B, j, s2, c)
    kn = ey * y
    # rows (h, s2, c) with h in {0,1} duplicated; cols j
    kd_t = np.tile(ey.transpose(0, 2, 3, 1).reshape(B, S * CIN, N), (1, 2, 1))
    kn_t = np.tile(kn.transpose(0, 2, 3, 1).reshape(B, S * CIN, N), (1, 2, 1))

    bd = np.zeros((128, 128), np.float32)
    for pl in range(16):
        for g in range(GDIM):
            for c in range(CIN):
                bd[pl * GDIM + g, pl * CIN + c] = w_g[c, g]

    sind = np.zeros((128, 16), np.float32)
    for h in range(2):
        for s2 in range(S):
            for c in range(CIN):
                sind[h * 64 + s2 * CIN + c, h * CIN + c] = 1.0

    consts_plane = np.empty((N, CONSTW), np.float32)
    consts_plane[:, IDENT0:IDENT0 + 128] = np.eye(128, dtype=np.float32)
    consts_plane[:, KD0:KD0 + B * N] = kd_t.transpose(1, 0, 2).reshape(128, -1)
    consts_plane[:, KN0:KN0 + B * N] = kn_t.transpose(1, 0, 2).reshape(128, -1)
    consts_plane[:, BD0:BD0 + 128] = bd
    consts_plane[:, SIND0:SIND0 + 16] = sind

    in_maps = []
    for k in range(NCORES):
        sl = slice(ISHARD * k, ISHARD * (k + 1))
        pg_core = pairwise_g[:, sl].reshape(NBLK, N, BW)
        blob = np.empty((N, TOTW), np.float32)
        blob[:, :CONSTW] = consts_plane
        blob[:, CONSTW:] = pg_core.transpose(1, 0, 2).reshape(N, NBLK * BW)
        in_maps.append({"blob": blob})
    return in_maps


def _host_finish(s_list, coset_functions, mask, w_lin):
    """Decode per-core (16, 512) outputs into the full result."""
    y = np.asarray(coset_functions, dtype=np.float32)
    maskf = np.asarray(mask).astype(np.float32)
    out = np.empty((B, N, S, COUT), np.float32)
    for k in range(NCORES):
        s = s_list[k]
        den = s[:, :NCOL].reshape(2, CIN, NBLK, NQ)
        num = s[:, NCOL:].reshape(2, CIN, NBLK, NQ)
        # (h, c, blk, q) -> (blk, s1 = 2q + h, c)
        den = den.transpose(2, 3, 0, 1).reshape(NBLK, S, CIN)
        num = num.transpose(2, 3, 0, 1).reshape(NBLK, S, CIN)
        sl = slice(ISHARD * k, ISHARD * (k + 1))
        y_q = y[:, sl].reshape(NBLK, S, CIN)
        m_q = maskf[:, sl].reshape(NBLK, S)
        res = (y_q + num / den) * m_q[..., None]
        res = res @ w_lin.T
        out[:, sl] = res.reshape(B, ISHARD, S, COUT)
    return out


def kernel(pairwise_g, coset_functions, mask, w_y, b_y, w_g, b_g, w_lin):
    pairwise_g = np.asarray(pairwise_g, dtype=np.float32)
    coset_functions = np.asarray(coset_functions, dtype=np.float32)
    mask = np.asarray(mask)
    w_y = np.asarray(w_y, dtype=np.float32)
    w_g = np.asarray(w_g, dtype=np.float32)
    w_lin = np.asarray(w_lin, dtype=np.float32)

    nc = _get_program()
    in_maps = _host_prep(pairwise_g, coset_functions, mask, w_y, w_g)
    res = run_bass_kernel_spmd(nc, in_maps, core_ids=list(range(NCORES)))
    s_list = [r["out_s"] for r in res.results]
    return _host_finish(s_list, coset_functions, mask, w_lin)


# revision 43
# speedup vs baseline: 2.4886x; 2.4886x over previous
"""Trainium2 Bass kernel for equivariant multihead attention.

Math (per batch b, query point i, coset s1, channel c):
    logit[j,s2] = sum_g pairwise_g[b,i,j,s1,s2,g]*w_g[c,g]
                  + w_y[c,0]*y[b,j,s2,c] + w_y[c,1]*y[b,i,s1,c] + b_g[c] + b_y[c]
    att = exp(logit)*mask[b,j,s2];  att /= sum_{j,s2} att
    out = (y[b,i,s1,c] + sum_{j,s2} att*y[b,j,s2,c]) * mask[b,i,s1]  @ w_lin.T

The query-side term and the biases are constant over the key dims (j,s2), so
they cancel in the normalization and are dropped.  The key-side factor
exp(w_y[c,0]*y[b,j,s2,c])*mask[b,j,s2] is a tiny per-batch table KD (and
KD*y = KN), precomputed on host.  Per (b,i) block the device computes
    E[(s1,s2,c), j] = exp(sum_g G_T[(s1,s2,g), j] * w_g[c,g])
    den_part[(s1,s2,c)] = sum_j E * KD_T     (fused multiply-reduce)
    num_part[(s1,s2,c)] = sum_j E * KN_T
and one final PE matmul sums the partials over s2.  Host finishes with the
residual add, query mask, and the c_in->c_out linear (all tiny).

Sharding: query dim i is split 8 ways (16 i x 4 b = 64 blocks per core).

Implementation notes (walrus on this stack allows only ONE sync wait per
Matmult / DMA / STT instruction, and ~12 on the final drain):
  * ALL inputs ship as ONE dram "blob" per core, loaded by 7 big
    column-range DMAs into a single SBUF plane -> every DMA is the first
    on its HW-DGE queue (no proc-predecessor wait) and descriptor runs are
    ~17KB contiguous (max DMA efficiency).  The final store is the 8th DMA
    (queue 7, also virgin).
  * tiny "spacer" ops make each engine observe cross-engine ticks ahead of
    the real instructions, so those carry at most one wait each.
"""

import numpy as np

import concourse.bacc as bacc
import concourse.tile as tile
from concourse import mybir
from concourse.bass_utils import run_bass_kernel_spmd

B, N, S, CIN, COUT, GDIM = 4, 128, 8, 8, 8, 7
NCORES = 8
ISHARD = N // NCORES          # 16 query points per core
NBLK = B * ISHARD             # 64 (b,i) blocks per core
PW = S * GDIM                 # 56: free width of one s1 slice
QW = 2 * PW                   # 112: free width of one transpose quarter
NQ = 4                        # quarters per block
BW = NQ * QW                  # 448 floats per (j, block)
NCOL = NBLK * NQ              # 256 partial columns per half

# blob column layout: [ident | kd | kn | bd | sind | pg blocks]
IDENT0 = 0
KD0 = 128
KN0 = KD0 + B * N             # 640
BD0 = KN0 + B * N             # 1152
SIND0 = BD0 + 128             # 1280
CONSTW = SIND0 + 16           # 1296
TOTW = CONSTW + NBLK * BW     # 29968

# blocks covered by each of the 7 input DMAs (first also carries consts;
# earlier ones smaller for a faster pipeline ramp)
SUPER_BLOCKS = (4, 6, 8, 10, 11, 12, 13)

F32 = mybir.dt.float32

_PROGRAM_CACHE = {}


def _build_program(nblk=NBLK, loop_reps=1):
    """loop_reps>1 wraps the main loop in a hardware For_i that re-runs the
    full pass (including the input DMAs) on the same data -- used only for
    timing: wall(loop_reps=R) - wall(loop_reps=1) isolates device time from
    the ~100ms axon dispatch/transfer overhead."""
    nc = bacc.Bacc("TRN2", target_bir_lowering=False, debug=False,
                   num_devices=NCORES)

    blob_d = nc.dram_tensor("blob", (N, TOTW), F32, kind="ExternalInput").ap()
    out_s = nc.dram_tensor("out_s", (16, 2 * NCOL), F32,
                           kind="ExternalOutput").ap()

    # per-super [start_block, end_block) and column ranges
    supers = []
    blk0 = 0
    for nb in SUPER_BLOCKS:
        if blk0 >= nblk:
            break
        nb = min(nb, nblk - blk0)
        c0 = 0 if blk0 == 0 else CONSTW + blk0 * BW
        c1 = CONSTW + (blk0 + nb) * BW
        supers.append((blk0, blk0 + nb, c0, c1))
        blk0 += nb

    with tile.TileContext(nc) as tc:
        with (
            tc.tile_pool(name="consts", bufs=1) as consts,
            tc.tile_pool(name="gtpool", bufs=3) as gtpool,
            tc.tile_pool(name="epool", bufs=3) as epool,
            tc.tile_pool(name="psA", bufs=3, space="PSUM") as psA,
            tc.tile_pool(name="psB", bufs=2, space="PSUM") as psB,
            tc.tile_pool(name="psC", bufs=1, space="PSUM") as psC,
            tc.tile_pool(name="psJ", bufs=1, space="PSUM") as psJ,
        ):
            g_all = consts.tile([N, TOTW], F32)
            ident = g_all[:, IDENT0:IDENT0 + 128]
            bd = g_all[0:QW, BD0:BD0 + 128]
            sind = g_all[:, SIND0:SIND0 + 16]

            buf_dve = consts.tile([128, 2 * NCOL], F32)
            nc.vector.memset(buf_dve, 0.0)
            junk_d0 = consts.tile([128, 1], F32)

            NDUM = 8
            dummies = [consts.tile([128, 1], F32, name=f"dum{i}")
                       for i in range(NDUM)]
            dum_idx = [0]
            junk_sp = consts.tile([128, NBLK], F32)
            fence_buf = consts.tile([128, 2, NQ * NBLK], F32)
            junk_act = consts.tile([128, NBLK], F32)
            junk_act2 = consts.tile([128, NBLK], F32)
            s_sb = consts.tile([16, 2 * NCOL], F32)

            junk_ps = psJ.tile([1, 128], F32)

            def stt_reduce(e_q, table, col_ap):
                dum = dummies[dum_idx[0] % NDUM]
                dum_idx[0] += 1
                nc.vector.scalar_tensor_tensor(
                    dum.broadcast_to(e_q.shape), e_q, 0.0, table,
                    op0=mybir.AluOpType.bypass, op1=mybir.AluOpType.mult,
                    accum_out=col_ap)

            def main_pass():
              for (b0, b1, c0, c1) in supers:
                nc.sync.dma_start(g_all[:, c0:c1], blob_d[:, c0:c1])
              # DVE observes the super-0 queue (kd/kn/consts) once
              nc.vector.tensor_copy(junk_d0, g_all[:, KD0:KD0 + 1])
              for (b0, b1, c0, c1) in supers:
                # PE observes this super's DMA queue once; real transposes
                # then carry only their PSUM-slot WAR wait.
                nc.tensor.transpose(junk_ps, g_all[:, c0:c0 + 1], ident)
                for blk in range(b0, b1):
                    b = blk // ISHARD
                    gcol = CONSTW + blk * BW
                    kd_b = g_all[:, KD0 + b * N:KD0 + (b + 1) * N]
                    kn_b = g_all[:, KN0 + b * N:KN0 + (b + 1) * N]

                    gt_cat = gtpool.tile([QW, NQ, 128], F32, tag="gt")
                    for q in range(NQ):
                        gt_ps = psA.tile([QW, 128], F32, tag="gtps")
                        nc.tensor.transpose(
                            gt_ps,
                            g_all[:, gcol + QW * q:gcol + QW * (q + 1)],
                            ident)
                        nc.scalar.copy(gt_cat[:, q, :], gt_ps)

                    l_ps = psB.tile([128, NQ, 128], F32, tag="lps")
                    nc.tensor.matmul(l_ps, lhsT=bd, rhs=gt_cat,
                                     start=True, stop=True)
                    # ACT self-spacer: emits the {ACT >= copies(blk)} tick so
                    # the block reusing this gt_cat slot later carries only
                    # its {PE} wait.
                    nc.scalar.copy(junk_act2[:QW, blk:blk + 1],
                                   gt_cat[:, NQ - 1, 127:128])

                    e_t = epool.tile([128, NQ, 128], F32, tag="e")
                    nc.scalar.activation(e_t, l_ps,
                                         mybir.ActivationFunctionType.Exp)
                    # DVE spacer: absorbs the {ACT exp} wait for the stts
                    nc.vector.tensor_copy(junk_sp[:, blk:blk + 1],
                                          e_t[:, 0, 0:1])

                    for q in range(NQ):
                        col = blk * NQ + q
                        e_q = e_t[:, q, :]
                        stt_reduce(e_q, kd_b, buf_dve[:, col:col + 1])
                        stt_reduce(e_q, kn_b,
                                   buf_dve[:, NCOL + col:NCOL + col + 1])

                    # fence: data-ordered after all 8 accum writes; the ACT
                    # read of it makes ACT observe the DVE tick, so the next
                    # exp reusing this e-slot has no DVE WAR wait.
                    bcols = buf_dve.rearrange("p (h c) -> p h c", h=2)
                    nc.vector.tensor_copy(
                        fence_buf[:, :, NQ * blk:NQ * (blk + 1)],
                        bcols[:, :, NQ * blk:NQ * (blk + 1)])
                    nc.scalar.copy(junk_act[:, blk:blk + 1],
                                   fence_buf[:, 0, NQ * blk:NQ * blk + 1])

            if loop_reps > 1:
                with tc.For_i(0, loop_reps, 1,
                              hint_engines=(mybir.EngineType.PE,
                                            mybir.EngineType.Activation,
                                            mybir.EngineType.DVE,
                                            mybir.EngineType.SP)):
                    main_pass()
            else:
                main_pass()

            # sum the (h,s2,c) j-partials over s2 -> (h,c)
            s_ps = psC.tile([16, 2 * NCOL], F32)
            nc.tensor.matmul(s_ps, lhsT=sind, rhs=buf_dve,
                             start=True, stop=True)
            nc.scalar.copy(s_sb, s_ps)
            nc.sync.dma_start(out_s, s_sb)   # 8th DMA -> virgin queue 7

    nc.compile()   # bacc: register alloc + split_sync_waits (1-wait limit)
    return nc


def _get_program(nblk=NBLK, loop_reps=1):
    key = ("nc", nblk, loop_reps)
    if key not in _PROGRAM_CACHE:
        _PROGRAM_CACHE[key] = _build_program(nblk, loop_reps)
    return _PROGRAM_CACHE[key]


def _host_prep(pairwise_g, coset_functions, mask, w_y, w_g):
    """Build the per-core input blobs."""
    y = coset_functions.astype(np.float32)          # (B, N, S, C) keys
    maskf = mask.astype(np.float32)
    ey = np.exp(y * w_y[:, 0]) * maskf[..., None]   # (B, j, s2, c)
    kn = ey * y
    # rows (h, s2, c) with h in {0,1} duplicated; cols j
    kd_t = np.tile(ey.transpose(0, 2, 3, 1).reshape(B, S * CIN, N), (1, 2, 1))
    kn_t = np.tile(kn.transpose(0, 2, 3, 1).reshape(B, S * CIN, N), (1, 2, 1))

    bd = np.zeros((128, 128), np.float32)
    for pl in range(16):
        for g in range(GDIM):
            for c in range(CIN):
                bd[pl * GDIM + g, pl * CIN + c] = w_g[c, g]

    sind = np.zeros((128, 16), np.float32)
    for h in range(2):
        for s2 in range(S):
            for c in range(CIN):
                sind[h * 64 + s2 * CIN + c, h * CIN + c] = 1.0

    consts_plane = np.empty((N, CONSTW), np.float32)
    consts_plane[:, IDENT0:IDENT0 + 128] = np.eye(128, dtype=np.float32)
    consts_plane[:, KD0:KD0 + B * N] = kd_t.transpose(1, 0, 2).reshape(128, -1)
    consts_plane[:, KN0:KN0 + B * N] = kn_t.transpose(1, 0, 2).reshape(128, -1)
    consts_plane[:, BD0:BD0 + 128] = bd
    consts_plane[:, SIND0:SIND0 + 16] = sind

    in_maps = []
    for k in range(NCORES):
        sl = slice(ISHARD * k, ISHARD * (k + 1))
        pg_core = pairwise_g[:, sl].reshape(NBLK, N, BW)
        blob = np.empty((N, TOTW), np.float32)
        blob[:, :CONSTW] = consts_plane
        blob[:, CONSTW:] = pg_core.transpose(1, 0, 2).reshape(N, NBLK * BW)
        in_maps.append({"blob": blob})
    return in_maps


def _host_finish(s_list, coset_functions, mask, w_lin):
    """Decode per-core (16, 512) outputs into the full result."""
    y = np.asarray(coset_functions, dtype=np.float32)
    maskf = np.asarray(mask).astype(np.float32)
    out = np.empty((B, N, S, COUT), np.float32)
    for k in range(NCORES):
        s = s_list[k]
        den = s[:, :NCOL].reshape(2, CIN, NBLK, NQ)
        num = s[:, NCOL:].reshape(2, CIN, NBLK, NQ)
        # (h, c, blk, q) -> (blk, s1 = 2q + h, c)
        den = den.transpose(2, 3, 0, 1).reshape(NBLK, S, CIN)
        num = num.transpose(2, 3, 0, 1).reshape(NBLK, S, CIN)
        sl = slice(ISHARD * k, ISHARD * (k + 1))
        y_q = y[:, sl].reshape(NBLK, S, CIN)
        m_q = maskf[:, sl].reshape(NBLK, S)
        res = (y_q + num / den) * m_q[..., None]
        res = res @ w_lin.T
        out[:, sl] = res.reshape(B, ISHARD, S, COUT)
    return out


def kernel(pairwise_g, coset_functions, mask, w_y, b_y, w_g, b_g, w_lin):
    pairwise_g = np.asarray(pairwise_g, dtype=np.float32)
    coset_functions = np.asarray(coset_functions, dtype=np.float32)
    mask = np.asarray(mask)
    w_y = np.asarray(w_y, dtype=np.float32)
    w_g = np.asarray(w_g, dtype=np.float32)
    w_lin = np.asarray(w_lin, dtype=np.float32)

    nc = _get_program()
    in_maps = _host_prep(pairwise_g, coset_functions, mask, w_y, w_g)
    res = run_bass_kernel_spmd(nc, in_maps, core_ids=list(range(NCORES)))
    s_list = [r["out_s"] for r in res.results]
    return _host_finish(s_list, coset_functions, mask, w_lin)
